# revision 4
# baseline (speedup 1.0000x reference)
"""Single-head causal attention (B=8, T=2048, D=512, H=64) on 8 TRN2 cores.

Data-parallel: one batch element per NeuronCore. Each core computes
attention in the S^T layout (keys on partitions, queries on the free axis):

  qT/kT/vT [64, T] = W.T @ x.T        (f32r matmuls, N=512 chunks)
  v        [T, 64] via PE transpose of vT, with a ones column appended
  S^T[j,i] = kT_jblock.T @ qT          (strips of causal width)
  P^T      = exp(S^T / 8)              (ScalarE, one op per strip;
                                        no max-subtraction: scores are
                                        bounded by ~|q||k|sqrt(H)/8 << 88)
  out^T[h,i], l[i] = [v|1]_jb.T @ P^T  (accumulated over j-blocks in PSUM;
                                        row 64 is the softmax denominator)

The kernel returns the unnormalized [65, T] strip per core; the host
divides by the denominator row and transposes back to [T, 64].
"""

import sys

sys.path.insert(0, "/opt/trn_rl_repo")

import numpy as np

import concourse.bass as bass
import concourse.mybir as mybir
import concourse.tile as tile

B, T, D, H = 8, 2048, 512, 64
N_CORES = 8
HALF = T // 2  # i-axis pass width

f32 = mybir.dt.float32
f32r = mybir.dt.float32r

_cache = {}


def _legalize_waits(nc, max_waits=1):
    """Walrus codegen accepts at most one sync wait per instruction; hoist
    extras onto same-engine NOPs placed immediately before (engine queues
    are FIFO so blocking semantics are unchanged)."""
    counter = 0
    for bb in nc.main_func.blocks:
        if not any(
            ins.sync_info is not None and len(ins.sync_info.on_wait) > max_waits
            for ins in bb.instructions
        ):
            continue
        new_list = []
        for ins in bb.instructions:
            si = ins.sync_info
            if si is not None and len(si.on_wait) > max_waits:
                waits = list(si.on_wait)
                hoist, keep = waits[:-max_waits], waits[-max_waits:]
                for w in hoist:
                    counter += 1
                    new_list.append(
                        mybir.InstNoOp(
                            name=f"I-waitfix-{counter}",
                            engine=ins.engine,
                            sync_info=mybir.SyncInfo(on_wait=[w], on_update=[]),
                            bass_nofuse=True,
                        )
                    )
                ins.sync_info = mybir.SyncInfo(
                    on_wait=keep, on_update=list(si.on_update)
                )
            new_list.append(ins)
        bb.instructions = new_list
    return counter


def _chunks(lo, hi, step, align):
    """Split [lo, hi) at multiples of `step` relative to `align`."""
    out = []
    cur = lo
    while cur < hi:
        nxt = min(hi, align + ((cur - align) // step + 1) * step)
        out.append((cur, nxt))
        cur = nxt
    return out


def _build():
    nc = bass.Bass()

    xt_d = nc.declare_dram_parameter("xt", [D, T], f32r, isOutput=False)
    wqk_d = nc.declare_dram_parameter("wqk", [D, 2 * H], f32r, isOutput=False)
    wv_d = nc.declare_dram_parameter("wv", [D, H], f32r, isOutput=False)
    mask_d = nc.declare_dram_parameter("mask", [128, 128], f32r, isOutput=False)
    ident_d = nc.declare_dram_parameter("ident", [H, H], f32r, isOutput=False)
    ones_d = nc.declare_dram_parameter("ones", [128, T // 128], f32r, isOutput=False)
    out_d = nc.declare_dram_parameter("out", [H + 1, T], f32, isOutput=True)

    NC_TILES = D // 128  # 4 c-tiles

    with tile.TileContext(nc) as tc:
        with (
            tc.tile_pool(name="const", bufs=1) as cpool,
            tc.tile_pool(name="xt", bufs=1) as xpool,
            tc.tile_pool(name="qkv", bufs=1) as qkvpool,
            tc.tile_pool(name="p", bufs=2) as ppool,
            tc.tile_pool(name="o", bufs=2) as opool,
            tc.tile_pool(name="ps_proj", bufs=1, space="PSUM") as ps_proj,
            tc.tile_pool(name="ps_vt", bufs=1, space="PSUM") as ps_vt,
            tc.tile_pool(name="ps_s", bufs=2, space="PSUM") as ps_s,
            tc.tile_pool(name="ps_pv", bufs=1, space="PSUM") as ps_pv,
        ):
            wqk = cpool.tile([128, NC_TILES, 2 * H], f32r)
            wv = cpool.tile([128, NC_TILES, H], f32r)
            mask = cpool.tile([128, 128], f32r)
            ident = cpool.tile([H, H], f32r)
            for c in range(NC_TILES):
                nc.sync.dma_start(wqk[:, c, :], wqk_d[128 * c : 128 * (c + 1), :])
                nc.sync.dma_start(wv[:, c, :], wv_d[128 * c : 128 * (c + 1), :])
            nc.sync.dma_start(mask[:], mask_d[:])
            nc.sync.dma_start(ident[:], ident_d[:])

            xt = [
                xpool.tile([128, T], f32r, name=f"xt{c}") for c in range(NC_TILES)
            ]
            qT = qkvpool.tile([H, T], f32r)
            kT = qkvpool.tile([H, T], f32r)
            vT = qkvpool.tile([H, T], f32r)
            v1 = qkvpool.tile([128, T // 128, H + 1], f32r)
            nc.sync.dma_start(v1[:, :, H : H + 1], ones_d[:, :])

            for h in range(2):
                t0 = h * HALF
                for c in range(NC_TILES):
                    nc.sync.dma_start(
                        xt[c][:, t0 : t0 + HALF],
                        xt_d[128 * c : 128 * (c + 1), t0 : t0 + HALF],
                    )

                # --- projections for this half ---
                for tc512 in range(t0, t0 + HALF, 512):
                    qk_ps = ps_proj.tile([128, 512], f32, tag="proj")
                    for c in range(NC_TILES):
                        nc.tensor.matmul(
                            qk_ps[:],
                            wqk[:, c, :],
                            xt[c][:, tc512 : tc512 + 512],
                            start=(c == 0),
                            stop=(c == NC_TILES - 1),
                        )
                    nc.vector.tensor_copy(
                        qT[:, tc512 : tc512 + 512], qk_ps[0:H, :]
                    )
                    nc.vector.tensor_copy(
                        kT[:, tc512 : tc512 + 512], qk_ps[H : 2 * H, :]
                    )
                    v_ps = ps_proj.tile([128, 512], f32, tag="proj")
                    for c in range(NC_TILES):
                        nc.tensor.matmul(
                            v_ps[0:H, :],
                            wv[:, c, :],
                            xt[c][:, tc512 : tc512 + 512],
                            start=(c == 0),
                            stop=(c == NC_TILES - 1),
                        )
                    nc.vector.tensor_copy(vT[:, tc512 : tc512 + 512], v_ps[0:H, :])

                # --- v transposes: vT [64, T] -> v1 [j, h] tiles ---
                vt_ps = ps_vt.tile([128, 8, H], f32r, tag="vt")
                for jl, jj in enumerate(range(8 * h, 8 * h + 8)):
                    nc.tensor.transpose(
                        vt_ps[:, jl, :], vT[:, 128 * jj : 128 * (jj + 1)], ident[:]
                    )
                    nc.vector.tensor_copy(v1[:, jj, 0:H], vt_ps[:, jl, :])

                # --- attention pass over this half of i ---
                pv_ps = ps_pv.tile([H + 1, HALF], f32, tag="pv")
                n_jb = 8 * h + 8
                for jb in range(n_jb):
                    i_start = max(t0, 128 * jb)
                    W = t0 + HALF - i_start
                    s_ps = ps_s.tile([128, HALF], f32, tag="s")
                    # S^T strip: chunk by strip-local 512 (PSUM bank) bounds
                    for ls, le in _chunks(0, W, 512, 0):
                        nc.tensor.matmul(
                            s_ps[:, ls:le],
                            kT[:, 128 * jb : 128 * (jb + 1)],
                            qT[:, i_start + ls : i_start + le],
                            start=True,
                            stop=True,
                        )
                    p_sb = ppool.tile([128, HALF], f32r, tag="p")
                    nc.scalar.activation(
                        p_sb[:, 0:W],
                        s_ps[:, 0:W],
                        mybir.ActivationFunctionType.Exp,
                        scale=1.0 / 8.0,
                    )
                    if 128 * jb >= t0:
                        nc.vector.tensor_mul(
                            p_sb[:, 0:128], p_sb[:, 0:128], mask[:]
                        )
                    # PV accumulate: chunk by global-512 (pv bank) bounds
                    for gs, ge in _chunks(i_start, t0 + HALF, 512, 0):
                        ic_last_jb = min(n_jb - 1, (ge - 1) // 128)
                        nc.tensor.matmul(
                            pv_ps[:, gs - t0 : ge - t0],
                            v1[:, jb, :],
                            p_sb[:, gs - i_start : ge - i_start],
                            start=(jb == 0),
                            stop=(jb == ic_last_jb),
                        )
                out_sb = opool.tile([H + 1, HALF], f32, tag="o")
                nc.scalar.copy(out_sb[:], pv_ps[:])
                nc.sync.dma_start(out_d[:, t0 : t0 + HALF], out_sb[:])

    _legalize_waits(nc)
    return nc


def build_in_maps(x, Wq, Wk, Wv):
    x = np.ascontiguousarray(np.asarray(x), dtype=np.float32)
    wqk_np = np.ascontiguousarray(
        np.concatenate([np.asarray(Wq), np.asarray(Wk)], axis=1), dtype=np.float32
    )
    wv_np = np.ascontiguousarray(np.asarray(Wv), dtype=np.float32)
    # keep iff j_local <= i_local (upper-triangular in [j, i] layout)
    mask_np = np.triu(np.ones((128, 128), dtype=np.float32))
    ident_np = np.eye(H, dtype=np.float32)
    ones_np = np.ones((128, T // 128), dtype=np.float32)
    return [
        {
            "xt": np.ascontiguousarray(x[b].T),
            "wqk": wqk_np,
            "wv": wv_np,
            "mask": mask_np,
            "ident": ident_np,
            "ones": ones_np,
        }
        for b in range(N_CORES)
    ]


def kernel(x, Wq, Wk, Wv):
    from concourse.bass_utils import run_bass_kernel_spmd

    if "nc" not in _cache:
        _cache["nc"] = _build()
    nc = _cache["nc"]

    in_maps = build_in_maps(x, Wq, Wk, Wv)
    res = run_bass_kernel_spmd(nc, in_maps, list(range(N_CORES))).results

    out = np.empty((B, T, H), dtype=np.float32)
    for b in range(N_CORES):
        strip = res[b]["out"]  # [H+1, T]
        out[b] = (strip[:H, :] / strip[H : H + 1, :]).T
    return out


if __name__ == "__main__":
    rng = np.random.default_rng(0)
    x = rng.standard_normal((B, T, D)).astype(np.float32)
    s = 1.0 / np.sqrt(D)
    Wq = (rng.standard_normal((D, H)) * s).astype(np.float32)
    Wk = (rng.standard_normal((D, H)) * s).astype(np.float32)
    Wv = (rng.standard_normal((D, H)) * s).astype(np.float32)
    out = kernel(x=x, Wq=Wq, Wk=Wk, Wv=Wv)
    print("out", out.shape, out.dtype, np.abs(out).max())
